# revision 27
# baseline (speedup 1.0000x reference)
"""Per-entity linear head: out[n, e] = sum_h x[n, e, h] * W[e, h] + b[e].

Full inputs: cell_states (4, 512, 64, 1024) f32, W (64, 1024), b (64,).
Data-parallel over the flattened batch*seq dim across 8 cores (64 MiB of
x per core); W/b are tiny and replicated, host-duplicated to 128
partitions so no on-chip broadcast is ever needed.

Per core: x_core viewed as [16384, 1024] rows.  Reduce-tile tt puts row
128*tt + p on partition p, so partition p always owns entity
e = p % 64 and W needs only a [128, 1024] resident tile.  One fused DVE
scalar_tensor_tensor per tile computes y[:, tt] = sum_h(x * w) in a
single pass over the data (the elementwise product is discarded into a
stride-0 dummy); the bias is one per-partition tensor_scalar_add on the
final [128, 128] result, which is stored contiguously and untangled on
the host with a free numpy transpose.

The kernel is HBM-read-bound: ~333 GB/s/core is the measured DMA
ceiling here (64 MiB => ~202 us), DVE busy is ~156 us and hides under
the DMA stream.  DMA granularity: G=4 reduce-tiles (2 MiB) per
dma_start through the 16 HW queues; the last tiles are issued singly
(512 KiB) so the post-last-DMA compute tail is one STT, not four.

Notes:
- bacc.Bacc + nc.compile() (not raw Bass): compile() splits multi-sem
  waits into EventSemaphore instructions (walrus here allows only one
  wait per instruction) and codegens InstISA subclasses.
- The fused DVE TENSOR_TENSOR_REDUCE (InstISA) compiles but faults at
  runtime on this terminal; InstTensorScalarPtr (scalar_tensor_tensor)
  with accum_out is the native-BIR equivalent and runs fine.
"""

import numpy as np

import concourse.bass as bass
import concourse.mybir as mybir
from concourse import bacc, bass_utils
from concourse.tile import TileContext

B, S, E, H = 4, 512, 64, 1024
N_CORES = 8
N = B * S                # 2048 flattened batch*seq rows
NPC = N // N_CORES       # 256 n-rows per core
R = NPC * E              # 16384 (n, e) rows of length H per core
P = 128                  # SBUF partitions
T = R // P               # 128 reduce tiles / output columns per core
G = 8                    # reduce tiles per main DMA (4 MiB each)
TAIL_SINGLES = 0         # end taper measurably starves the DMA queues
                         # (GpSimd offload of tiles fails walrus codegen)
X_BUFS = 5


def build() -> bass.Bass:
    nc = bacc.Bacc("TRN2", target_bir_lowering=False, enable_asserts=False)
    x = nc.dram_tensor("x", [R, H], mybir.dt.float32, kind="ExternalInput")
    w = nc.dram_tensor("w", [P, H], mybir.dt.float32, kind="ExternalInput")
    bvec = nc.dram_tensor("bvec", [P, 1], mybir.dt.float32, kind="ExternalInput")
    y = nc.dram_tensor("y", [P, T], mybir.dt.float32, kind="ExternalOutput")

    xt_rows = x.rearrange("(tt p) h -> tt p h", p=P)  # [T, P, H]

    # (start_tile, ntiles) chunks: big G-tile groups, then single-tile
    # chunks at the end so the post-last-DMA compute tail is one STT.
    # (Tapering the *start* was tried and hurts: fragmenting the head of
    # the DMA stream costs more than the earlier compute start saves.)
    chunks = []
    tt = 0
    while tt < T - TAIL_SINGLES:
        n = min(G, T - TAIL_SINGLES - tt)
        chunks.append((tt, n))
        tt += n
    while tt < T:
        chunks.append((tt, 1))
        tt += 1

    with TileContext(nc) as tc:
        with (
            tc.tile_pool(name="xpool", bufs=X_BUFS) as xpool,
            tc.tile_pool(name="consts", bufs=1) as consts,
            tc.tile_pool(name="wpsum", bufs=1, space="PSUM") as wpsum,
            # scratch (dummy product sink) stays in SBUF: putting it in
            # PSUM contends with the w reads on DVE's PSUM port (+5 us)
            tc.tile_pool(name="scratch", bufs=4) as scratch,
        ):
            # w lives in PSUM: the DVE reads it over its dedicated PSUM
            # port, halving DVE's SBUF read traffic (which contends with
            # the 370 GB/s DMA write stream).  DMA can't target PSUM, so
            # stage through SBUF and copy on the otherwise-idle ScalarE.
            w_stage = consts.tile([P, H], mybir.dt.float32)
            w_sb = wpsum.tile([P, H], mybir.dt.float32)
            b_sb = consts.tile([P, 1], mybir.dt.float32)
            y_sb = consts.tile([P, T], mybir.dt.float32)

            # w/b first (tiny, ~1.3 us): the SBUF->PSUM copy overlaps the
            # first x chunk's DMA so the first STT starts as soon as the
            # chunk lands
            nc.sync.dma_start(out=w_stage[:], in_=w[:])
            nc.scalar.copy(w_sb[:], w_stage[:])
            nc.sync.dma_start(out=b_sb[:], in_=bvec[:])

            for start, ntiles in chunks:
                xt = xpool.tile([P, ntiles, H], mybir.dt.float32, tag="xt")
                nc.sync.dma_start(
                    out=xt[:],
                    in_=xt_rows[start : start + ntiles].rearrange("t p h -> p t h"),
                )
                for i in range(ntiles):
                    c = start + i
                    dummy = scratch.tile([P, 1], mybir.dt.float32)
                    nc.vector.scalar_tensor_tensor(
                        out=dummy.broadcast_to((P, H)),
                        in0=xt[:, i],
                        scalar=1.0,
                        in1=w_sb[:],
                        op0=mybir.AluOpType.mult,
                        op1=mybir.AluOpType.mult,
                        accum_out=y_sb[:, c : c + 1],
                    )
            # y += b (per-partition scalar), then store the result
            nc.vector.tensor_scalar_add(y_sb[:], y_sb[:], b_sb[:, 0:1])
            nc.sync.dma_start(out=y[:], in_=y_sb[:])
    nc.compile()
    return nc


def _prepare_in_maps(cell_states, W, b):
    x_all = np.ascontiguousarray(cell_states, dtype=np.float32).reshape(N * E, H)
    w2 = np.ascontiguousarray(np.concatenate([W, W], axis=0), dtype=np.float32)
    b2 = np.ascontiguousarray(
        np.concatenate([b, b]).reshape(P, 1), dtype=np.float32
    )
    in_maps = []
    for c in range(N_CORES):
        xc = x_all[c * R : (c + 1) * R]
        in_maps.append({"x": xc, "w": w2, "bvec": b2})
    return in_maps


def _unshard(per_core_y):
    outs = []
    for y_raw in per_core_y:
        # y_raw[p, tt] = out[2*tt + p//64, p%64] within the core's 256 rows
        outs.append(
            np.asarray(y_raw).reshape(2, E, T).transpose(2, 0, 1).reshape(NPC, E)
        )
    return np.concatenate(outs, axis=0).reshape(B, S, E)


def kernel_with_results(trace=False, **inputs):
    nc = build()
    in_maps = _prepare_in_maps(inputs["cell_states"], inputs["W"], inputs["b"])
    res = bass_utils.run_bass_kernel_spmd(
        nc, in_maps, core_ids=list(range(N_CORES)), trace=trace
    )
    out = _unshard([r["y"] for r in res.results])
    return out, res


def kernel(**inputs) -> np.ndarray:
    out, _ = kernel_with_results(trace=False, **inputs)
    return out
